# revision 5
# baseline (speedup 1.0000x reference)
"""GQA attention (B=2, S=2048, D=2048, Hq=16, Hkv=4, hd=128) on 8 TRN2 cores.

Sharding: core c = b*4 + kv handles batch b and kv-head kv (with its 4 query
heads). Each core computes its partial output (A_heads @ Wo_slice); the host
sums the 4 partials per batch and adds the bias.

All-bf16 instruction mix with HW-tuned scheduling:
- DMA ordered by first use (wk/wv chunks, x block 0 per-chunk, wq, x b1, wo,
  ...); host pre-lays weights as [P, DC, F] so each loads in contiguous runs.
- softmax denominator as two pair-sum chains (early half on Pool mid-loop,
  late half on DVE, s2+t0 precombined at j==14); the 1s-broadcast matmul pd
  issues at j==11 of the next head (in-order PE must never park on it), then
  reciprocal + normalize on DVE.
- out-projection/deferred-Q fillers popped from a global FIFO by cycle
  budget; h2/h3 matmuls gated at slot start+17+gi//2 (later gates head-block
  the queue and starve the PE of pacing work -> p-state drops).
- engine balance: per-head drains and pass-A psum-drain copies split
  ACT/DVE (ACT idle otherwise); ACT exp table preloaded during pass A.
- rejected on HW measurement: fp8 DoubleRow (weight-load bound at the
  512-col psum cap), gpsimd partition_all_reduce (~9us/call), bf16 output.
"""
import sys

sys.path.insert(0, "/opt/trn_rl_repo")
import numpy as np

B, S, D = 2, 2048, 2048
HQ, HKV, HD = 16, 4, 128
G = HQ // HKV
SCALE = HD ** -0.5
P = 128
NB = 512
DC = D // P
SB = S // NB
ST = S // P
DEPTH = 6

_CACHE = {}


def _build(reps=(1, 1, 1)):
    from contextlib import ExitStack, nullcontext

    import concourse.bacc as bacc
    import concourse.mybir as mybir
    import concourse.tile as tile
    from concourse.masks import make_identity

    F32 = mybir.dt.float32
    BF16 = mybir.dt.bfloat16
    Exp = mybir.ActivationFunctionType.Exp
    Copy = mybir.ActivationFunctionType.Copy
    MULT = mybir.AluOpType.mult

    nc = bacc.Bacc("TRN2", target_bir_lowering=False, debug=False)
    xd = nc.dram_tensor("xb16", [P, DC, S], BF16, kind="ExternalInput").ap()
    wqd = nc.dram_tensor("wqb", [P, DC, G * HD], BF16, kind="ExternalInput").ap()
    wkd = nc.dram_tensor("wkb", [P, DC, HD], BF16, kind="ExternalInput").ap()
    wvd = nc.dram_tensor("wvb", [P, DC, HD], BF16, kind="ExternalInput").ap()
    wod = nc.dram_tensor("wob", [P, G, D], BF16, kind="ExternalInput").ap()
    out = nc.dram_tensor("out", [S, D], F32, kind="ExternalOutput").ap()

    r1, r2, _ = reps

    with tile.TileContext(nc) as tc, ExitStack() as stk:
        persist = stk.enter_context(tc.tile_pool(name="persist", bufs=1))
        kt_sb = persist.tile([P, S], BF16)
        v_sb = persist.tile([P, ST, HD], BF16)
        qt_sb = persist.tile([P, G, S], BF16)
        xt3 = persist.tile([P, DC, NB], BF16)
        wq_sb = persist.tile([P, DC, G * HD], BF16)
        wk_sb = persist.tile([P, DC, HD], BF16)
        wv_sb = persist.tile([P, DC, HD], BF16)
        wo_sb = persist.tile([P, G, D], BF16)
        ident = persist.tile([P, P], BF16)
        ones = persist.tile([P, P], BF16)
        make_identity(nc, ident)
        nc.gpsimd.memset(ones, 1.0)

        nc.sync.dma_start(out=wk_sb, in_=wkd)
        nc.sync.dma_start(out=wv_sb, in_=wvd)

        def _loop(r):
            return tc.For_i(0, r, 1) if r > 1 else nullcontext()

        # ---- pass A ----
        with ExitStack() as pas:
            xta_pool = pas.enter_context(tc.tile_pool(name="xta", bufs=2))
            vt_pool = pas.enter_context(tc.tile_pool(name="vt", bufs=2))
            ps_a = pas.enter_context(tc.tile_pool(name="ps_a", bufs=1, space="PSUM"))
            ps_t = pas.enter_context(tc.tile_pool(name="ps_t", bufs=2, space="PSUM"))

            with _loop(r1):
              for xb in range(SB):
                cols = slice(xb * NB, (xb + 1) * NB)
                xt = xt3 if xb == SB - 1 else xta_pool.tile(
                    [P, DC, NB], BF16, name="xt")
                for qi in range(4):
                    nc.sync.dma_start(out=xt[:, 4 * qi:4 * qi + 4, :],
                                      in_=xd[:, 4 * qi:4 * qi + 4, cols])
                    if xb == 0 and qi == 3:
                        nc.sync.dma_start(out=wq_sb, in_=wqd)
                nheads = 0 if xb == SB - 1 else G
                pk = ps_a.tile([P, NB], F32, name="pk")
                pv = ps_a.tile([P, NB], F32, name="pv")
                pqs = [ps_a.tile([P, NB], F32, name=f"pq{h}")
                       for h in range(nheads)]
                for c in range(DC):
                    nc.tensor.matmul(pk, wk_sb[:, c, :], xt[:, c, :],
                                     start=(c == 0), stop=(c == DC - 1))
                for c in range(DC):
                    nc.tensor.matmul(pv, wv_sb[:, c, :], xt[:, c, :],
                                     start=(c == 0), stop=(c == DC - 1))
                for h in range(nheads):
                    hsl = slice(h * HD, (h + 1) * HD)
                    for c in range(DC):
                        nc.tensor.matmul(pqs[h], wq_sb[:, c, hsl], xt[:, c, :],
                                         start=(c == 0), stop=(c == DC - 1))
                nc.vector.tensor_copy(out=kt_sb[:, cols], in_=pk)
                vt = vt_pool.tile([P, NB], BF16, name="vt")
                nc.vector.tensor_copy(out=vt, in_=pv)
                for h in range(nheads):
                    nc.vector.tensor_copy(out=qt_sb[:, h, cols], in_=pqs[h])
                if xb == 0:
                    nc.sync.dma_start(out=wo_sb, in_=wod)
                for k in range(NB // P):
                    pt = ps_t.tile([P, P], BF16, name="pt")
                    nc.tensor.transpose(pt, vt[:, k * P:(k + 1) * P], ident)
                    nc.vector.tensor_copy(out=v_sb[:, xb * (NB // P) + k, :],
                                          in_=pt)

        # ---- pass B ----
        with ExitStack() as pbs:
            ex_pool = pbs.enter_context(tc.tile_pool(name="ex", bufs=3))
            ot_pool = pbs.enter_context(tc.tile_pool(name="otp", bufs=3))
            dn_pool = pbs.enter_context(tc.tile_pool(name="dn", bufs=2))
            st_pool = pbs.enter_context(tc.tile_pool(name="st", bufs=6))
            ps_st = pbs.enter_context(tc.tile_pool(name="ps_st", bufs=2, space="PSUM"))
            ps_po = pbs.enter_context(tc.tile_pool(name="ps_po", bufs=1, space="PSUM"))
            ps_aux = pbs.enter_context(tc.tile_pool(name="ps_aux", bufs=2, space="PSUM"))
            ps_pd = pbs.enter_context(tc.tile_pool(name="ps_pd", bufs=1, space="PSUM"))

            with _loop(r2):
                ot_tiles = {}
                aux_state = {}

                def q3_fillers():
                    fillers = []
                    icols3 = slice((SB - 1) * NB, SB * NB)
                    for h in range(G):
                        hsl = slice(h * HD, (h + 1) * HD)
                        for c in range(DC):
                            def go(h=h, hsl=hsl, c=c):
                                if c == 0:
                                    aux_state["q"] = ps_aux.tile(
                                        [P, NB], F32, name="aux")
                                pq = aux_state["q"]
                                nc.tensor.matmul(pq, wq_sb[:, c, hsl],
                                                 xt3[:, c, :],
                                                 start=(c == 0),
                                                 stop=(c == DC - 1))
                                if c == DC - 1:
                                    nc.vector.tensor_copy(
                                        out=qt_sb[:, h, icols3], in_=pq)
                            fillers.append((0, 512, go))
                    return fillers

                def outproj_fillers(ib, start, tail=False):
                    fillers = []
                    ot = ot_tiles[ib]
                    groups = [(t, nb) for t in range(4) for nb in range(D // NB)]
                    for gi, (t, nb) in enumerate(groups):
                        tsl = slice(t * P, (t + 1) * P)
                        nsl = slice(nb * NB, (nb + 1) * NB)
                        for h in range(G):
                            ms = start if h < 2 else start + 17 + gi // 2
                            def go(gi=gi, t=t, nb=nb, tsl=tsl, nsl=nsl, h=h,
                                   ib=ib, ot=ot, tail=tail):
                                key = ("o", gi % 2)
                                if h == 0:
                                    aux_state[key] = ps_aux.tile(
                                        [P, NB], F32, name="aux")
                                pso = aux_state[key]
                                nc.tensor.matmul(pso, ot[:, h, tsl],
                                                 wo_sb[:, h, nsl],
                                                 start=(h == 0), stop=(h == 3))
                                if h == 3:
                                    so = st_pool.tile([P, NB], F32, name="so")
                                    if tail and gi % 2 == 0:
                                        nc.scalar.activation(
                                            out=so, in_=pso, func=Copy)
                                    else:
                                        nc.vector.tensor_copy(out=so, in_=pso)
                                    it = 4 * ib + t
                                    nc.sync.dma_start(
                                        out=out[it * P:(it + 1) * P, nsl],
                                        in_=so)
                            fillers.append((ms, 512, go))
                    return fillers

                pending_tail = [None]

                def head_tail(ib, h, dsum, oun):
                    def go():
                        pd = ps_pd.tile([P, NB], F32, name="pd")
                        nc.tensor.matmul(pd, ones, dsum, start=True, stop=True)
                        rec = dn_pool.tile([P, NB], F32, name="rec")
                        nc.vector.reciprocal(out=rec, in_=pd)
                        nc.vector.tensor_tensor(out=ot_tiles[ib][:, h, :],
                                                in0=oun, in1=rec, op=MULT)
                    return go

                filler_q = []
                cur_slot = [0]

                def pop_fillers(budget=512):
                    while filler_q:
                        ms, cost, fn = filler_q[0]
                        if ms > cur_slot[0] or cost > budget:
                            break
                        filler_q.pop(0)
                        fn()
                        budget -= cost

                for ib in range(SB):
                    icols = slice(ib * NB, (ib + 1) * NB)
                    ot_tiles[ib] = ot_pool.tile([P, G, NB], BF16, name="ot")
                    if ib == 0:
                        filler_q += q3_fillers()
                    else:
                        filler_q += outproj_fillers(ib - 1, cur_slot[0])
                    for h in range(G):
                        ex = ex_pool.tile([P, ST, NB], BF16, name="ex")
                        po = ps_po.tile([P, NB], F32, name="po")
                        ca = dn_pool.tile([P, 2, NB], BF16, name="ca")
                        t0 = dn_pool.tile([P, 2, NB], BF16, name="t0")
                        s2 = dn_pool.tile([P, 2, NB], BF16, name="s2")
                        pss = None
                        for j in range(ST + DEPTH):
                            if j < ST:
                                if j % 2 == 0:
                                    pss = ps_st.tile([P, 2, NB], F32, name="pss")
                                nc.tensor.matmul(pss[:, j % 2, :],
                                                 kt_sb[:, j * P:(j + 1) * P],
                                                 qt_sb[:, h, icols],
                                                 start=True, stop=True)
                                if j % 2 == 1:
                                    nc.scalar.activation(
                                        out=ex[:, j - 1:j + 1, :], in_=pss,
                                        func=Exp, scale=SCALE)
                            jc = j - DEPTH
                            if jc >= 0:
                                nc.tensor.matmul(po, v_sb[:, jc, :],
                                                 ex[:, jc, :],
                                                 start=(jc == 0),
                                                 stop=(jc == ST - 1))
                            if j == 11 and pending_tail[0] is not None:
                                pending_tail[0]()
                                pending_tail[0] = None
                            if j == 5:
                                nc.gpsimd.tensor_add(out=ca, in0=ex[:, 0:2, :],
                                                     in1=ex[:, 2:4, :])
                            elif j == 9:
                                nc.gpsimd.tensor_add(out=t0, in0=ex[:, 4:6, :],
                                                     in1=ex[:, 6:8, :])
                            elif j == 11:
                                nc.gpsimd.tensor_add(out=t0, in0=t0, in1=ca)
                            elif j == 13:
                                nc.vector.tensor_add(out=s2,
                                                     in0=ex[:, 8:10, :],
                                                     in1=ex[:, 10:12, :])
                            pop_fillers()
                            cur_slot[0] += 1
                        s3 = dn_pool.tile([P, 2, NB], BF16, name="s3")
                        nc.vector.tensor_add(out=s3, in0=ex[:, 12:14, :],
                                             in1=ex[:, 14:16, :])
                        nc.vector.tensor_add(out=s3, in0=s3, in1=s2)
                        nc.vector.tensor_add(out=s3, in0=s3, in1=t0)
                        dsum = dn_pool.tile([P, NB], BF16, name="dsum")
                        nc.vector.tensor_add(out=dsum, in0=s3[:, 0, :],
                                             in1=s3[:, 1, :])
                        oun = dn_pool.tile([P, NB], BF16, name="oun")
                        nc.vector.tensor_copy(out=oun, in_=po)
                        pending_tail[0] = head_tail(ib, h, dsum, oun)
                cur_slot[0] += 10 ** 6
                pop_fillers(10 ** 9)
                pending_tail[0]()
                pending_tail[0] = None
                for ms, cost, fn in outproj_fillers(SB - 1, 0, tail=True):
                    fn()

    nc.compile()
    return nc


def _get_nc():
    if "nc" not in _CACHE:
        _CACHE["nc"] = _build()
    return _CACHE["nc"]


def timed_runner(reps):
    nc = _build(reps)
    return make_runner(nc)


def make_runner(nc, n_cores=8):
    """Persistent jitted SPMD runner (mirrors bass2jax.run_bass_via_pjrt's
    multi-core path, without donation so the executable can be re-invoked on
    device-resident inputs for timing)."""
    import jax
    from jax.experimental.shard_map import shard_map
    from jax.sharding import Mesh, PartitionSpec

    import concourse.mybir as mybir
    from concourse import bass2jax

    bass2jax.install_neuronx_cc_hook()
    partition_name = nc.partition_id_tensor.name if nc.partition_id_tensor else None
    in_names, out_names, out_avals, zero_shapes = [], [], [], []
    for alloc in nc.m.functions[0].allocations:
        if not isinstance(alloc, mybir.MemoryLocationSet):
            continue
        name = alloc.memorylocations[0].name
        if alloc.kind == "ExternalInput":
            if name != partition_name:
                in_names.append(name)
        elif alloc.kind == "ExternalOutput":
            out_names.append(name)
            shape = tuple(alloc.tensor_shape)
            dtype = mybir.dt.np(alloc.dtype)
            out_avals.append(jax.core.ShapedArray(shape, dtype))
            zero_shapes.append((shape, dtype))
    n_params = len(in_names)
    all_in_names = tuple(in_names + out_names)
    if partition_name is not None:
        all_in_names = all_in_names + (partition_name,)

    def _body(*args):
        operands = list(args)
        if partition_name is not None:
            operands.append(bass2jax.partition_id_tensor())
        outs = bass2jax._bass_exec_p.bind(
            *operands,
            out_avals=tuple(out_avals),
            in_names=all_in_names,
            out_names=tuple(out_names),
            lowering_input_output_aliases=(),
            sim_require_finite=True,
            sim_require_nnan=True,
            nc=nc,
        )
        return tuple(outs)

    devices = jax.devices()[:n_cores]
    mesh = Mesh(np.asarray(devices), ("core",))
    n_outs = len(out_names)
    fn = jax.jit(
        shard_map(_body, mesh=mesh,
                  in_specs=(PartitionSpec("core"),) * (n_params + n_outs),
                  out_specs=(PartitionSpec("core"),) * n_outs,
                  check_rep=False),
        keep_unused=True,
    )
    return fn, in_names, out_names, zero_shapes, mesh




def _get_runner():
    if "runner" not in _CACHE:
        _CACHE["runner"] = make_runner(_get_nc())
    return _CACHE["runner"]


def run_cores(in_maps):
    import jax  # noqa: F401

    fn, in_names, out_names, zero_shapes, mesh = _get_runner()
    n = len(in_maps)
    concat_in = [np.concatenate([np.asarray(in_maps[c][nm]) for c in range(n)],
                                axis=0) for nm in in_names]
    concat_zero = [np.zeros((n * s[0], *s[1:]), dt) for s, dt in zero_shapes]
    outs = fn(*concat_in, *concat_zero)
    outs = [np.asarray(o) for o in outs]
    return [
        {nm: outs[i].reshape(n, *zero_shapes[i][0])[c]
         for i, nm in enumerate(out_names)}
        for c in range(n)
    ]


def shard_inputs(x, Wq, Wk, Wv, Wo):
    import ml_dtypes

    bf16 = ml_dtypes.bfloat16

    def lay(w):
        d0, f = w.shape
        return np.ascontiguousarray(
            w.reshape(d0 // P, P, f).transpose(1, 0, 2).astype(bf16))

    in_maps = []
    for b in range(B):
        xl = lay(np.ascontiguousarray(x[b].T))
        for kv in range(HKV):
            in_maps.append({
                "xb16": xl,
                "wqb": lay(np.ascontiguousarray(
                    Wq[:, kv * G * HD:(kv + 1) * G * HD])),
                "wkb": lay(np.ascontiguousarray(Wk[:, kv * HD:(kv + 1) * HD])),
                "wvb": lay(np.ascontiguousarray(Wv[:, kv * HD:(kv + 1) * HD])),
                "wob": lay(np.ascontiguousarray(
                    Wo[kv * G * HD:(kv + 1) * G * HD, :])),
            })
    return in_maps


def kernel(x, Wq, Wk, Wv, Wo, bo):
    x = np.asarray(x, np.float32)
    Wq = np.asarray(Wq, np.float32)
    Wk = np.asarray(Wk, np.float32)
    Wv = np.asarray(Wv, np.float32)
    Wo = np.asarray(Wo, np.float32)
    bo = np.asarray(bo, np.float32)
    results = run_cores(shard_inputs(x, Wq, Wk, Wv, Wo))
    out = np.empty((B, S, D), np.float32)
    for b in range(B):
        out[b] = results[4 * b]["out"]
        for kv in range(1, HKV):
            out[b] += results[4 * b + kv]["out"]
        out[b] += bo
    return out


# revision 6
# speedup vs baseline: 1.1908x; 1.1908x over previous
"""GQA attention (B=2, S=2048, D=2048, Hq=16, Hkv=4, hd=128) on 8 TRN2 cores.

Sharding: core c = b*4 + kv handles batch b and kv-head kv (with its 4 query
heads). Each core computes its partial output (A_heads @ Wo_slice); the host
sums the 4 partials per batch and adds the bias.

All-bf16 instruction mix with HW-tuned scheduling:
- DMA ordered by first use (wk/wv chunks, x block 0 per-chunk, wq, x b1, wo,
  ...); host pre-lays weights as [P, DC, F] so each loads in contiguous runs.
- softmax denominator as two pair-sum chains (early half on Pool mid-loop,
  late half on DVE, s2+t0 precombined at j==14); the 1s-broadcast matmul pd
  issues at j==11 of the next head (in-order PE must never park on it), then
  reciprocal + normalize on DVE.
- out-projection/deferred-Q fillers popped from a global FIFO by cycle
  budget; h2/h3 matmuls gated at slot start+17+gi//2 (later gates head-block
  the queue and starve the PE of pacing work -> p-state drops).
- engine balance: per-head drains and pass-A psum-drain copies split
  ACT/DVE (ACT idle otherwise); ACT exp table preloaded during pass A.
- rejected on HW measurement: fp8 DoubleRow (weight-load bound at the
  512-col psum cap), gpsimd partition_all_reduce (~9us/call), bf16 output.
"""
import sys

sys.path.insert(0, "/opt/trn_rl_repo")
import numpy as np

B, S, D = 2, 2048, 2048
HQ, HKV, HD = 16, 4, 128
G = HQ // HKV
SCALE = HD ** -0.5
P = 128
NB = 512
DC = D // P
SB = S // NB
ST = S // P
DEPTH = 6

_CACHE = {}


def _build(reps=(1, 1, 1)):
    from contextlib import ExitStack, nullcontext

    import concourse.bacc as bacc
    import concourse.mybir as mybir
    import concourse.tile as tile
    from concourse.masks import make_identity

    F32 = mybir.dt.float32
    BF16 = mybir.dt.bfloat16
    Exp = mybir.ActivationFunctionType.Exp
    Copy = mybir.ActivationFunctionType.Copy
    MULT = mybir.AluOpType.mult

    nc = bacc.Bacc("TRN2", target_bir_lowering=False, debug=False)
    xd = nc.dram_tensor("xb16", [P, DC, S], BF16, kind="ExternalInput").ap()
    wqd = nc.dram_tensor("wqb", [P, DC, G * HD], BF16, kind="ExternalInput").ap()
    wkd = nc.dram_tensor("wkb", [P, DC, HD], BF16, kind="ExternalInput").ap()
    wvd = nc.dram_tensor("wvb", [P, DC, HD], BF16, kind="ExternalInput").ap()
    wod = nc.dram_tensor("wob", [P, G, D], BF16, kind="ExternalInput").ap()
    out = nc.dram_tensor("out", [S, D], F32, kind="ExternalOutput").ap()

    r1, r2, _ = reps

    with tile.TileContext(nc) as tc, ExitStack() as stk:
        persist = stk.enter_context(tc.tile_pool(name="persist", bufs=1))
        kt_sb = persist.tile([P, S], BF16)
        v_sb = persist.tile([P, ST, HD], BF16)
        qt_sb = persist.tile([P, G, S], BF16)
        xt3 = persist.tile([P, DC, NB], BF16)
        wq_sb = persist.tile([P, DC, G * HD], BF16)
        wk_sb = persist.tile([P, DC, HD], BF16)
        wv_sb = persist.tile([P, DC, HD], BF16)
        wo_sb = persist.tile([P, G, D], BF16)
        ident = persist.tile([P, P], BF16)
        ones = persist.tile([P, P], BF16)
        make_identity(nc, ident)
        nc.gpsimd.memset(ones, 1.0)

        nc.sync.dma_start(out=wk_sb, in_=wkd)
        nc.sync.dma_start(out=wv_sb, in_=wvd)

        def _loop(r):
            return tc.For_i(0, r, 1) if r > 1 else nullcontext()

        # ---- pass A ----
        with ExitStack() as pas:
            xta_pool = pas.enter_context(tc.tile_pool(name="xta", bufs=2))
            vt_pool = pas.enter_context(tc.tile_pool(name="vt", bufs=2))
            ps_a = pas.enter_context(tc.tile_pool(name="ps_a", bufs=1, space="PSUM"))
            ps_t = pas.enter_context(tc.tile_pool(name="ps_t", bufs=2, space="PSUM"))

            with _loop(r1):
              for xb in range(SB):
                cols = slice(xb * NB, (xb + 1) * NB)
                xt = xt3 if xb == SB - 1 else xta_pool.tile(
                    [P, DC, NB], BF16, name="xt")
                for qi in range(4):
                    nc.sync.dma_start(out=xt[:, 4 * qi:4 * qi + 4, :],
                                      in_=xd[:, 4 * qi:4 * qi + 4, cols])
                    if xb == 0 and qi == 3:
                        nc.sync.dma_start(out=wq_sb, in_=wqd)
                nheads = 0 if xb == SB - 1 else G
                pk = ps_a.tile([P, NB], F32, name="pk")
                pv = ps_a.tile([P, NB], F32, name="pv")
                pqs = [ps_a.tile([P, NB], F32, name=f"pq{h}")
                       for h in range(nheads)]
                for c in range(DC):
                    nc.tensor.matmul(pk, wk_sb[:, c, :], xt[:, c, :],
                                     start=(c == 0), stop=(c == DC - 1))
                for c in range(DC):
                    nc.tensor.matmul(pv, wv_sb[:, c, :], xt[:, c, :],
                                     start=(c == 0), stop=(c == DC - 1))
                for h in range(nheads):
                    hsl = slice(h * HD, (h + 1) * HD)
                    for c in range(DC):
                        nc.tensor.matmul(pqs[h], wq_sb[:, c, hsl], xt[:, c, :],
                                         start=(c == 0), stop=(c == DC - 1))
                nc.vector.tensor_copy(out=kt_sb[:, cols], in_=pk)
                vt = vt_pool.tile([P, NB], BF16, name="vt")
                nc.vector.tensor_copy(out=vt, in_=pv)
                for h in range(nheads):
                    nc.vector.tensor_copy(out=qt_sb[:, h, cols], in_=pqs[h])
                if xb == 0:
                    nc.sync.dma_start(out=wo_sb, in_=wod)
                for k in range(NB // P):
                    pt = ps_t.tile([P, P], BF16, name="pt")
                    nc.tensor.transpose(pt, vt[:, k * P:(k + 1) * P], ident)
                    nc.vector.tensor_copy(out=v_sb[:, xb * (NB // P) + k, :],
                                          in_=pt)

        # ---- pass B ----
        with ExitStack() as pbs:
            ex_pool = pbs.enter_context(tc.tile_pool(name="ex", bufs=3))
            ot_pool = pbs.enter_context(tc.tile_pool(name="otp", bufs=2))
            dn_pool = pbs.enter_context(tc.tile_pool(name="dn", bufs=2))
            st_pool = pbs.enter_context(tc.tile_pool(name="st", bufs=6))
            ps_st = pbs.enter_context(tc.tile_pool(name="ps_st", bufs=2, space="PSUM"))
            ps_po = pbs.enter_context(tc.tile_pool(name="ps_po", bufs=1, space="PSUM"))
            ps_aux = pbs.enter_context(tc.tile_pool(name="ps_aux", bufs=2, space="PSUM"))
            ps_pd = pbs.enter_context(tc.tile_pool(name="ps_pd", bufs=1, space="PSUM"))

            with _loop(r2):
                ot_tiles = {}
                aux_state = {}

                def q3_fillers():
                    fillers = []
                    icols3 = slice((SB - 1) * NB, SB * NB)
                    for h in range(G):
                        hsl = slice(h * HD, (h + 1) * HD)
                        for c in range(DC):
                            def go(h=h, hsl=hsl, c=c):
                                if c == 0:
                                    aux_state["q"] = ps_aux.tile(
                                        [P, NB], F32, name="aux")
                                pq = aux_state["q"]
                                nc.tensor.matmul(pq, wq_sb[:, c, hsl],
                                                 xt3[:, c, :],
                                                 start=(c == 0),
                                                 stop=(c == DC - 1))
                                if c == DC - 1:
                                    nc.vector.tensor_copy(
                                        out=qt_sb[:, h, icols3], in_=pq)
                            fillers.append((0, 512, go))
                    return fillers

                def outproj_fillers(ib, start, tail=False):
                    fillers = []
                    ot = ot_tiles[ib]
                    groups = [(t, nb) for t in range(4) for nb in range(D // NB)]
                    for gi, (t, nb) in enumerate(groups):
                        tsl = slice(t * P, (t + 1) * P)
                        nsl = slice(nb * NB, (nb + 1) * NB)
                        for h in range(G):
                            ms = start if h < 2 else start + 17 + gi // 2
                            def go(gi=gi, t=t, nb=nb, tsl=tsl, nsl=nsl, h=h,
                                   ib=ib, ot=ot, tail=tail):
                                key = ("o", gi % 2)
                                if h == 0:
                                    aux_state[key] = ps_aux.tile(
                                        [P, NB], F32, name="aux")
                                pso = aux_state[key]
                                nc.tensor.matmul(pso, ot[:, h, tsl],
                                                 wo_sb[:, h, nsl],
                                                 start=(h == 0), stop=(h == 3))
                                if h == 3:
                                    so = st_pool.tile([P, NB], F32, name="so")
                                    if tail and gi % 2 == 0:
                                        nc.scalar.activation(
                                            out=so, in_=pso, func=Copy)
                                    else:
                                        nc.vector.tensor_copy(out=so, in_=pso)
                                    it = 4 * ib + t
                                    nc.sync.dma_start(
                                        out=out[it * P:(it + 1) * P, nsl],
                                        in_=so)
                            fillers.append((ms, 512, go))
                    return fillers

                pending_tail = [None]

                def head_tail(ib, h, dsum, oun):
                    def go():
                        pd = ps_pd.tile([P, NB], F32, name="pd")
                        nc.tensor.matmul(pd, ones, dsum, start=True, stop=True)
                        rec = dn_pool.tile([P, NB], F32, name="rec")
                        nc.vector.reciprocal(out=rec, in_=pd)
                        nc.vector.tensor_tensor(out=ot_tiles[ib][:, h, :],
                                                in0=oun, in1=rec, op=MULT)
                    return go

                filler_q = []
                cur_slot = [0]

                def pop_fillers(budget=512):
                    while filler_q:
                        ms, cost, fn = filler_q[0]
                        if ms > cur_slot[0] or cost > budget:
                            break
                        filler_q.pop(0)
                        fn()
                        budget -= cost

                for ib in range(SB):
                    icols = slice(ib * NB, (ib + 1) * NB)
                    ot_tiles[ib] = ot_pool.tile([P, G, NB], BF16, name="ot")
                    if ib == 0:
                        filler_q += q3_fillers()
                    else:
                        filler_q += outproj_fillers(ib - 1, cur_slot[0])
                    for h in range(G):
                        ex = ex_pool.tile([P, ST, NB], BF16, name="ex")
                        po = ps_po.tile([P, NB], F32, name="po")
                        ca = dn_pool.tile([P, 2, NB], BF16, name="ca")
                        t0 = dn_pool.tile([P, 2, NB], BF16, name="t0")
                        s2 = dn_pool.tile([P, 2, NB], BF16, name="s2")
                        pss = None
                        for j in range(ST + DEPTH):
                            if j < ST:
                                if j % 2 == 0:
                                    pss = ps_st.tile([P, 2, NB], F32, name="pss")
                                nc.tensor.matmul(pss[:, j % 2, :],
                                                 kt_sb[:, j * P:(j + 1) * P],
                                                 qt_sb[:, h, icols],
                                                 start=True, stop=True)
                                if j % 2 == 1:
                                    nc.scalar.activation(
                                        out=ex[:, j - 1:j + 1, :], in_=pss,
                                        func=Exp, scale=SCALE)
                            jc = j - DEPTH
                            if jc >= 0:
                                nc.tensor.matmul(po, v_sb[:, jc, :],
                                                 ex[:, jc, :],
                                                 start=(jc == 0),
                                                 stop=(jc == ST - 1))
                            if j == 11 and pending_tail[0] is not None:
                                pending_tail[0]()
                                pending_tail[0] = None
                            if j == 5:
                                nc.gpsimd.tensor_add(out=ca, in0=ex[:, 0:2, :],
                                                     in1=ex[:, 2:4, :])
                            elif j == 9:
                                nc.gpsimd.tensor_add(out=t0, in0=ex[:, 4:6, :],
                                                     in1=ex[:, 6:8, :])
                            elif j == 11:
                                nc.gpsimd.tensor_add(out=t0, in0=t0, in1=ca)
                            elif j == 13:
                                nc.vector.tensor_add(out=s2,
                                                     in0=ex[:, 8:10, :],
                                                     in1=ex[:, 10:12, :])
                            pop_fillers()
                            cur_slot[0] += 1
                        s3 = dn_pool.tile([P, 2, NB], BF16, name="s3")
                        nc.vector.tensor_add(out=s3, in0=ex[:, 12:14, :],
                                             in1=ex[:, 14:16, :])
                        nc.vector.tensor_add(out=s3, in0=s3, in1=s2)
                        nc.vector.tensor_add(out=s3, in0=s3, in1=t0)
                        dsum = dn_pool.tile([P, NB], BF16, name="dsum")
                        nc.vector.tensor_add(out=dsum, in0=s3[:, 0, :],
                                             in1=s3[:, 1, :])
                        oun = dn_pool.tile([P, NB], BF16, name="oun")
                        nc.vector.tensor_copy(out=oun, in_=po)
                        pending_tail[0] = head_tail(ib, h, dsum, oun)
                cur_slot[0] += 10 ** 6
                pop_fillers(10 ** 9)
                pending_tail[0]()
                pending_tail[0] = None
                for ms, cost, fn in outproj_fillers(SB - 1, 0, tail=True):
                    fn()

    nc.compile()
    return nc


def _get_nc():
    if "nc" not in _CACHE:
        _CACHE["nc"] = _build()
    return _CACHE["nc"]


def timed_runner(reps):
    nc = _build(reps)
    return make_runner(nc)


def make_runner(nc, n_cores=8):
    """Persistent jitted SPMD runner (mirrors bass2jax.run_bass_via_pjrt's
    multi-core path, without donation so the executable can be re-invoked on
    device-resident inputs for timing)."""
    import jax
    from jax.experimental.shard_map import shard_map
    from jax.sharding import Mesh, PartitionSpec

    import concourse.mybir as mybir
    from concourse import bass2jax

    bass2jax.install_neuronx_cc_hook()
    partition_name = nc.partition_id_tensor.name if nc.partition_id_tensor else None
    in_names, out_names, out_avals, zero_shapes = [], [], [], []
    for alloc in nc.m.functions[0].allocations:
        if not isinstance(alloc, mybir.MemoryLocationSet):
            continue
        name = alloc.memorylocations[0].name
        if alloc.kind == "ExternalInput":
            if name != partition_name:
                in_names.append(name)
        elif alloc.kind == "ExternalOutput":
            out_names.append(name)
            shape = tuple(alloc.tensor_shape)
            dtype = mybir.dt.np(alloc.dtype)
            out_avals.append(jax.core.ShapedArray(shape, dtype))
            zero_shapes.append((shape, dtype))
    n_params = len(in_names)
    all_in_names = tuple(in_names + out_names)
    if partition_name is not None:
        all_in_names = all_in_names + (partition_name,)

    def _body(*args):
        operands = list(args)
        if partition_name is not None:
            operands.append(bass2jax.partition_id_tensor())
        outs = bass2jax._bass_exec_p.bind(
            *operands,
            out_avals=tuple(out_avals),
            in_names=all_in_names,
            out_names=tuple(out_names),
            lowering_input_output_aliases=(),
            sim_require_finite=True,
            sim_require_nnan=True,
            nc=nc,
        )
        return tuple(outs)

    devices = jax.devices()[:n_cores]
    mesh = Mesh(np.asarray(devices), ("core",))
    n_outs = len(out_names)
    fn = jax.jit(
        shard_map(_body, mesh=mesh,
                  in_specs=(PartitionSpec("core"),) * (n_params + n_outs),
                  out_specs=(PartitionSpec("core"),) * n_outs,
                  check_rep=False),
        keep_unused=True,
    )
    return fn, in_names, out_names, zero_shapes, mesh




def _get_runner():
    if "runner" not in _CACHE:
        _CACHE["runner"] = make_runner(_get_nc())
    return _CACHE["runner"]


def run_cores(in_maps):
    import jax  # noqa: F401

    fn, in_names, out_names, zero_shapes, mesh = _get_runner()
    n = len(in_maps)
    concat_in = [np.concatenate([np.asarray(in_maps[c][nm]) for c in range(n)],
                                axis=0) for nm in in_names]
    concat_zero = [np.zeros((n * s[0], *s[1:]), dt) for s, dt in zero_shapes]
    outs = fn(*concat_in, *concat_zero)
    outs = [np.asarray(o) for o in outs]
    return [
        {nm: outs[i].reshape(n, *zero_shapes[i][0])[c]
         for i, nm in enumerate(out_names)}
        for c in range(n)
    ]


def shard_inputs(x, Wq, Wk, Wv, Wo):
    import ml_dtypes

    bf16 = ml_dtypes.bfloat16

    def lay(w):
        d0, f = w.shape
        return np.ascontiguousarray(
            w.reshape(d0 // P, P, f).transpose(1, 0, 2).astype(bf16))

    in_maps = []
    for b in range(B):
        xl = lay(np.ascontiguousarray(x[b].T))
        for kv in range(HKV):
            in_maps.append({
                "xb16": xl,
                "wqb": lay(np.ascontiguousarray(
                    Wq[:, kv * G * HD:(kv + 1) * G * HD])),
                "wkb": lay(np.ascontiguousarray(Wk[:, kv * HD:(kv + 1) * HD])),
                "wvb": lay(np.ascontiguousarray(Wv[:, kv * HD:(kv + 1) * HD])),
                "wob": lay(np.ascontiguousarray(
                    Wo[kv * G * HD:(kv + 1) * G * HD, :])),
            })
    return in_maps


def kernel(x, Wq, Wk, Wv, Wo, bo):
    x = np.asarray(x, np.float32)
    Wq = np.asarray(Wq, np.float32)
    Wk = np.asarray(Wk, np.float32)
    Wv = np.asarray(Wv, np.float32)
    Wo = np.asarray(Wo, np.float32)
    bo = np.asarray(bo, np.float32)
    results = run_cores(shard_inputs(x, Wq, Wk, Wv, Wo))
    out = np.empty((B, S, D), np.float32)
    for b in range(B):
        out[b] = results[4 * b]["out"]
        for kv in range(1, HKV):
            out[b] += results[4 * b + kv]["out"]
        out[b] += bo
    return out
